# revision 16
# baseline (speedup 1.0000x reference)
"""Trainium2 Bass kernel for capsule dynamic routing (nn_Capsule).

Reference computation:
    hat = (x @ kernel).reshape(B, N, C, D).transpose(0, 2, 1, 3)   # [B,C,N,D]
    b = 0; 3 routing iterations of:
        w = softmax(b, axis=capsules)
        o = squash(einsum('bcn,bcnd->bcd', w, hat))
        b += einsum('bcd,bcnd->bcn', o, hat)

Key reformulation (hat is never materialized):
    o[c,d]  = sum_i xw[c,i] * K[i,(c,d)]      xw = w[c,:] @ x      (u-pass)
    bupd[c,n] = sum_i x[n,i] * oK[c,i]        oK[c,i] = sum_d o[c,d]*K[i,(c,d)]

Sharding: data-parallel over batch B=16 across 8 cores (2 items/core fused
into the same matmuls as a 2-wide moving operand). K replicated per core in
both orientations (kk: i-on-partitions for the u-passes, kt: d-on-partitions
for the oK-passes).

Performance structure (v2): the kernel is jointly limited by the 9 MB/core
input DMA (~26 us at ~360 GB/s) and the PE weight-load stream (~740
LDWEIGHTS+MATMUL pairs at ~37 ns measured). kk/kt are DMA'd in interleaved
capsule blocks and iteration 0 computes just-in-time behind the stream
(u0/oK0 interleaved per block, bupd filling the DMA stalls); iterations 1-2
then run from SBUF-resident K at full PE rate. Routing logits live in PSUM
across iterations (iteration-1 bupd matmuls accumulate with start=False),
removing the logit copy/add from the critical path.
"""

import numpy as np
import ml_dtypes
from contextlib import ExitStack

NCORES = 8
B, N, DI = 16, 512, 256         # batch, input capsules, input dim
C, D = 64, 128                  # output capsules, capsule dim
NB = B // NCORES                # batch items per core
P = 128                         # SBUF partitions
NCH = N // P                    # 4 n-chunks
ICH = DI // P                   # 2 i-chunks
NBLK = 8                        # capsule blocks
BC = C // NBLK                  # capsules per block
HALF = C // 2
ROUTINGS = 3
EPS = 1e-7

_cache = {}


def _build_program(reps=0):
    """reps=0: plain single-shot program (graded path).
    reps>0: wrap the whole body (input DMA + compute + output DMA) in a
    For_i loop for wall-clock-difference benchmarking."""
    import concourse.bass_isa as bass_isa
    import concourse.mybir as mybir
    import concourse.tile as tile
    from concourse import bacc

    F32 = mybir.dt.float32
    BF16 = mybir.dt.bfloat16
    AF = mybir.ActivationFunctionType
    AX = mybir.AxisListType

    class _OneActSetBacc(bacc.Bacc):
        """Every activation func used here (Square/Ln/Exp/Copy) lives in the
        'natural_log_exp_and_others' table set, but the default chooser picks
        per-func sets greedily and flip-flops (one ~1.3us LoadActFuncSet per
        switch, on the critical path). Mask the other sets so exactly one
        table load is emitted; indices are preserved so act_func_set_id still
        points at the real act_info.json entry."""

        def insert_act_table_loads(self):
            import bass_rust as _br
            from concourse.hw_specs import get_activation_tables

            has_activation = any(
                isinstance(i, mybir.InstActivation)
                for b in self.main_func.blocks
                for i in b.instructions
            )
            if not has_activation:
                return
            tables = [
                (name, funcs if name == "natural_log_exp_and_others" else set())
                for name, funcs in get_activation_tables(self.m.arch).items()
            ]
            _br.insert_act_table_loads(self, tables)

    nc = _OneActSetBacc("TRN2", target_bir_lowering=False, debug=False)

    x_d = nc.dram_tensor("x", [P, NB, NCH, DI], BF16, kind="ExternalInput").ap()
    xT_d = nc.dram_tensor("xT", [P, NB, ICH, N], BF16, kind="ExternalInput").ap()
    kk_d = nc.dram_tensor("kk", [P, NBLK, ICH, BC * D], BF16,
                          kind="ExternalInput").ap()
    kt_d = nc.dram_tensor("kt", [P, NBLK, BC, DI], BF16,
                          kind="ExternalInput").ap()
    xbar_d = nc.dram_tensor("xbar", [P, ICH, NB], BF16, kind="ExternalInput").ap()
    # output is [d, (c,b)]; the host does the final transpose to [b, c, d]
    out_d = nc.dram_tensor("out", [P, C * NB], F32, kind="ExternalOutput").ap()

    with tile.TileContext(nc) as tc, ExitStack() as ctx:
        big = ctx.enter_context(tc.tile_pool(name="big", bufs=1))
        wk = ctx.enter_context(tc.tile_pool(name="wk", bufs=2))
        psU = ctx.enter_context(tc.tile_pool(name="psU", bufs=1, space="PSUM"))
        psK = ctx.enter_context(tc.tile_pool(name="psK", bufs=1, space="PSUM"))
        psB = ctx.enter_context(tc.tile_pool(name="psB", bufs=2, space="PSUM"))
        psX = ctx.enter_context(tc.tile_pool(name="psX", bufs=1, space="PSUM"))

        kk = big.tile([P, NBLK, ICH, BC * D], BF16)
        kt = big.tile([P, NBLK, BC, DI], BF16)
        xs = big.tile([P, NB, NCH, DI], BF16)
        xT = big.tile([P, NB, ICH, N], BF16)
        xbar = big.tile([P, ICH, NB], BF16)

        o_bf = big.tile([P, C, NB], BF16)       # current (unscaled) o
        xwT = big.tile([P, ICH, C, NB], BF16)
        epst = big.tile([P, 1], F32)
        nc.vector.memset(epst, EPS)
        zerot = big.tile([P, 1], F32)
        nc.vector.memset(zerot, 0.0)
        # Dummy activation up front so the one LoadActFuncSet (~1.3us) runs
        # during the initial DMA wait instead of on the critical path.
        warm = big.tile([P, 1], F32)
        nc.scalar.activation(out=warm, in_=zerot[:], func=AF.Exp, bias=zerot[:])

        def pb_tiles():
            # routing-logit update of one iteration, [n, nch, c] per batch
            # item; psB rotates (bufs=2) so iteration 0's logits stay live
            # and iteration 1's softmax sums both generations in one DVE op
            return [psB.tile([P, NCH, C], F32, tag=f"pb{b}", name=f"pb{b}")
                    for b in range(NB)]

        def emit_input_dmas():
            # One sync-queue stream ordered by first use: xbar feeds the first
            # u0 matmul, xT feeds bupd (which fills iteration-0 DMA stalls);
            # kk/kt interleave per capsule block so the u0/oK0 passes chase
            # the stream block by block; xs (first needed by xw after all of
            # iteration 0) rides in the middle.
            nc.sync.dma_start(out=xbar, in_=xbar_d)
            nc.sync.dma_start(out=xT, in_=xT_d)
            for blk in range(NBLK):
                nc.sync.dma_start(out=kk[:, blk], in_=kk_d[:, blk])
                nc.sync.dma_start(out=kt[:, blk], in_=kt_d[:, blk])
                if blk == 2:
                    nc.sync.dma_start(out=xs, in_=x_d)

        def u_blk(po, blk, rhs_fn):
            """po[d,(c,b)] += kk-block matmuls for one capsule block, plus the
            (unscaled) o_bf eviction that feeds the oK matmuls."""
            for c_ in range(BC):
                c = blk * BC + c_
                for t in range(ICH):
                    nc.tensor.matmul(
                        po[:, c, :],
                        lhsT=kk[:, blk, t, c_ * D:(c_ + 1) * D],
                        rhs=rhs_fn(t, c),
                        start=(t == 0),
                        stop=(t == ICH - 1),
                    )
            nc.vector.tensor_copy(
                out=o_bf[:, blk * BC:(blk + 1) * BC, :],
                in_=po[:, blk * BC:(blk + 1) * BC, :],
            )

        def oK_blk(pk, blk):
            for c_ in range(BC):
                c = blk * BC + c_
                for t in range(ICH):
                    nc.tensor.matmul(
                        pk[:, t, c, :],
                        lhsT=kt[:, blk, c_, t * P:(t + 1) * P],
                        rhs=o_bf[:, c, :],
                        start=True,
                        stop=True,
                    )

        def scale_half(po2, half, ch):
            """squash scale for one capsule half, in free layout [*, (c b)]
            (identical rows via the partition reduce) so it can be applied
            with free-dim broadcasts downstream. sqrt(t) = exp(0.5*ln(t)) so
            Ln/Exp/Square share one activation-table set. The scale commutes
            past the (linear) oK matmuls and is applied at oks eviction."""
            sq, S, num, den, scale = ch
            s = slice(half * HALF * NB, (half + 1) * HALF * NB)
            nc.scalar.activation(out=sq[:, s], in_=po2[:, s], func=AF.Square)
            nc.gpsimd.partition_all_reduce(S[:, s], sq[:, s], P,
                                           bass_isa.ReduceOp.add)
            nc.scalar.activation(out=num[:, s], in_=S[:, s], func=AF.Ln,
                                 bias=epst[:])
            nc.scalar.activation(out=num[:, s], in_=num[:, s], func=AF.Exp,
                                 scale=0.5, bias=zerot[:])
            nc.vector.tensor_scalar_add(den[:, s], S[:, s], 0.5 + EPS)
            nc.vector.reciprocal(den[:, s], den[:, s])
            nc.vector.tensor_mul(scale[:, s], num[:, s], den[:, s])

        def chain_tiles():
            sq = wk.tile([P, C * NB], F32, tag="sq", name="sq")
            S = wk.tile([P, C * NB], F32, tag="S", name="S")
            num = wk.tile([P, C * NB], F32, tag="num", name="num")
            den = wk.tile([P, C * NB], F32, tag="den", name="den")
            scale = wk.tile([P, C * NB], F32, tag="scalef", name="scalef")
            return sq, S, num, den, scale

        def oks_bupd_half(pb, pk, oks, sc3, half):
            """apply the squash scale during the PSUM->SBUF eviction of the
            oK results, then compute this iteration's routing-logit update
            in PSUM."""
            hs = slice(half * HALF, (half + 1) * HALF)
            for b in range(NB):
                for t in range(ICH):
                    nc.vector.tensor_tensor(
                        oks[:, b, t, hs], pk[:, t, hs, b], sc3[:, b, hs],
                        mybir.AluOpType.mult,
                    )
            for b in range(NB):
                for nt in range(NCH):
                    for t in range(ICH):
                        nc.tensor.matmul(
                            pb[b][:, nt, hs],
                            lhsT=xT[:, b, t, nt * P:(nt + 1) * P],
                            rhs=oks[:, b, t, hs],
                            start=(t == 0),
                            stop=(t == ICH - 1),
                        )

        def softmax_xw(pb, prev=None):
            """w = softmax over capsules of the logits (this iteration's PSUM
            update plus the previous iterations' SBUF-saved logits), then
            xwT[i,(c,b)] = sum_n x[n,i] w[n,c]; chained per batch item so the
            first xw matmuls start one chain-latency after the last bupd."""
            e = wk.tile([P, NB, NCH, C], F32, tag="e", name="e")
            es = wk.tile([P, NB, NCH, 1], F32, tag="es", name="es")
            w = wk.tile([P, NB, NCH, C], BF16, tag="w", name="w")
            if prev is not None:
                z = wk.tile([P, NB, NCH, C], F32, tag="z", name="z")
            px = [psX.tile([P, ICH, C], F32, tag=f"px{b}", name=f"px{b}")
                  for b in range(NB)]
            for b in range(NB):
                # values are O(1) so the softmax max-subtraction is unneeded
                if prev is not None:
                    nc.vector.tensor_tensor(
                        z[:, b], prev[:, b], pb[b][:],
                        mybir.AluOpType.add,
                    )
                    src = z[:, b]
                else:
                    src = pb[b][:]
                nc.scalar.activation(out=e[:, b], in_=src,
                                     func=AF.Exp, bias=zerot[:])
                nc.vector.reduce_sum(out=es[:, b], in_=e[:, b], axis=AX.X)
                nc.vector.reciprocal(es[:, b], es[:, b])
                nc.vector.tensor_tensor(
                    w[:, b], e[:, b],
                    es[:, b].to_broadcast((P, NCH, C)),
                    mybir.AluOpType.mult,
                )
                for t in range(ICH):
                    for ch in range(NCH):
                        nc.tensor.matmul(
                            px[b][:, t, :],
                            lhsT=xs[:, b, ch, t * P:(t + 1) * P],
                            rhs=w[:, b, ch],
                            start=(ch == 0),
                            stop=(ch == NCH - 1),
                        )
            for b in range(NB):
                for half in range(2):
                    hs = slice(half * HALF, (half + 1) * HALF)
                    nc.vector.tensor_copy(out=xwT[:, :, hs, b],
                                          in_=px[b][:, :, hs])

        def body():
            emit_input_dmas()

            # --- iteration 0: chase the DMA stream block by block ---
            po = psU.tile([P, C, NB], F32, tag="po", name="po")
            po2 = po[:].rearrange("p c b -> p (c b)")
            ch = chain_tiles()
            pk = psK.tile([P, ICH, C, NB], F32, tag="pk", name="pk")
            oks = wk.tile([P, NB, ICH, C], BF16, tag="oks", name="oks")
            sc3 = ch[4][:].rearrange("p (c b) -> p b c", b=NB)
            pb0 = pb_tiles()
            for half in range(2):
                for blk in range(half * (NBLK // 2), (half + 1) * (NBLK // 2)):
                    u_blk(po, blk, lambda t, c: xbar[:, t, :])
                    oK_blk(pk, blk)
                scale_half(po2, half, ch)
                oks_bupd_half(pb0, pk, oks, sc3, half)
            softmax_xw(pb0)
            # save iteration-0 logits to SBUF (off the critical path) so the
            # iteration-1 softmax can sum SBUF+PSUM in one DVE op
            bT = big.tile([P, NB, NCH, C], F32)
            for b in range(NB):
                nc.vector.tensor_copy(out=bT[:, b], in_=pb0[b][:])

            # --- iterations 1..: PE-bound, K resident in SBUF ---
            for it in range(1, ROUTINGS):
                po = psU.tile([P, C, NB], F32, tag="po", name="po")
                po2 = po[:].rearrange("p c b -> p (c b)")
                ch = chain_tiles()
                last = it == ROUTINGS - 1
                for half in range(2):
                    for blk in range(half * (NBLK // 2),
                                     (half + 1) * (NBLK // 2)):
                        u_blk(po, blk, lambda t, c: xwT[:, t, c, :])
                    scale_half(po2, half, ch)
                    if last:
                        # final squash: o = o_pre * scale, as [d, (c,b)]
                        s = slice(half * HALF * NB, (half + 1) * HALF * NB)
                        oout = wk.tile([P, C * NB], F32, tag="oout",
                                       name="oout")
                        nc.vector.tensor_mul(oout[:, s], po2[:, s],
                                             ch[4][:, s])
                        nc.sync.dma_start(out=out_d[:, s], in_=oout[:, s])
                if not last:
                    pk = psK.tile([P, ICH, C, NB], F32, tag="pk", name="pk")
                    oks = wk.tile([P, NB, ICH, C], BF16, tag="oks", name="oks")
                    sc3 = ch[4][:].rearrange("p (c b) -> p b c", b=NB)
                    pb1 = pb_tiles()
                    for blk in range(NBLK):
                        oK_blk(pk, blk)
                        if blk == NBLK // 2 - 1:
                            oks_bupd_half(pb1, pk, oks, sc3, 0)
                    oks_bupd_half(pb1, pk, oks, sc3, 1)
                    softmax_xw(pb1, prev=bT[:])

        if reps:
            with tc.For_i(0, reps, 1, hint_engines=(mybir.EngineType.PE,)):
                body()
        else:
            body()

    nc.compile()
    return nc


def _prep_inputs(x, kernel):
    bf16 = ml_dtypes.bfloat16
    # kk[p, blk, t, c_*D+d] = K[t*P+p, (blk*BC+c_)*D+d]
    kk = np.ascontiguousarray(
        kernel.reshape(ICH, P, NBLK, BC * D).transpose(1, 2, 0, 3)).astype(bf16)
    # kt[p, blk, c_, i] = K[i, (blk*BC+c_)*D+p]
    kt = np.ascontiguousarray(
        kernel.reshape(DI, NBLK, BC, D).transpose(3, 1, 2, 0)).astype(bf16)
    in_maps = []
    for s in range(NCORES):
        xc = x[s * NB:(s + 1) * NB]                      # [NB, N, DI]
        x_in = np.ascontiguousarray(
            xc.reshape(NB, NCH, P, DI).transpose(2, 0, 1, 3)).astype(bf16)
        xT_in = np.ascontiguousarray(
            xc.reshape(NB, N, ICH, P).transpose(3, 0, 2, 1)).astype(bf16)
        xb = xc.sum(axis=1) / C                          # [NB, DI] fp32
        xbar_in = np.ascontiguousarray(
            xb.reshape(NB, ICH, P).transpose(2, 1, 0)).astype(bf16)
        in_maps.append(
            {"x": x_in, "xT": xT_in, "kk": kk, "kt": kt, "xbar": xbar_in}
        )
    return in_maps


def kernel(x, kernel, _trace=False, _reps=0):
    from concourse.bass_utils import run_bass_kernel_spmd

    x = np.ascontiguousarray(np.asarray(x, dtype=np.float32))
    kernel = np.ascontiguousarray(np.asarray(kernel, dtype=np.float32))
    assert x.shape == (B, N, DI) and kernel.shape == (DI, C * D)

    key = ("nc", _reps)
    if key not in _cache:
        _cache[key] = _build_program(reps=_reps)
    nc = _cache[key]

    in_maps = _prep_inputs(x, kernel)
    res = run_bass_kernel_spmd(nc, in_maps, list(range(NCORES)), trace=_trace)
    _cache["last_result"] = res

    out = np.empty((B, C, D), dtype=np.float32)
    for s in range(NCORES):
        o = res.results[s]["out"]                        # [d, (c,b)]
        out[s * NB:(s + 1) * NB] = o.reshape(D, C, NB).transpose(2, 1, 0)
    return out
